# revision 8
# baseline (speedup 1.0000x reference)
"""Trainium2 Bass kernel for nn_DeepSetAttentionModel (segment_reduce) — v3.

v3 vs v2:
- All constants/weights packed host-side into TWO big DMAs (one f32
  pack, one bf16 pack) instead of ~45 small ones (the v2 trace showed a
  60us DMA-serialized pre-pass on the sync queue).
- Inputs loaded contiguously chunk-major ([128,128] tiles, 2 DMAs per
  tensor) and PE-transposed to token-major (strided 4B-gather DMAs were
  ~1.4us each).
- Emission interleaved: featurize runs 2 rows ahead of the MLP loop;
  attention tails (which contain Exp) run after all Sins -> still only
  2 act-table loads.
- Featurize fused with scalar_tensor_tensor + Sin(scale=-2pi) fold.
Everything else (token-major attention, chunk-stationary L3, ones-col
Z, rotating Act/DVE evacuations) as in v2.
"""
import numpy as np

B, T = 64, 4096
CH = 128
NPOS, V, NMOD = 16, 1, 15
PHI_IN = 32
DP, H = 64, 4
MAXTS = 100.0
NCORES = 8
NSLOTS = B // NCORES
TWOPI = 2.0 * np.pi
CMAXG = 32          # chunks per row in the packed input layout

# f32 pack column offsets (head = first HEADF cols, DMA'd first)
F_ID32, F_ITP8, F_IOTA15, F_IOTATOK = 0, 32, 40, 55
HEADF = 88
F_IDENTF, F_RATTW, F_RW2, F_RW3, F_WKAGG, F_WKX, F_WQT = 88, 216, 344, 472, 473, 729, 985
F_BD1, F_BP2, F_BP3, F_BS2, F_BS3, F_BRA, F_BR1, F_BR2 = 989, 990, 991, 992, 993, 994, 995, 996
F_BD2, F_BNR3 = 997, 998
NF = 999
HEADB = 128
# bf16 pack column offsets
G_IDENT, G_PW2, G_PW3, G_SW2, G_SW3, G_RW1 = 0, 128, 256, 384, 512, 640
G_PW1E, G_SW1E, G_DW1, G_DW2 = 1152, 1280, 1408, 1536
NB = 1568


def _build_nc(Cs, tile_mod, bass, mybir):
    f32 = mybir.dt.float32
    bf16 = mybir.dt.bfloat16
    i32 = mybir.dt.int32
    Alu = mybir.AluOpType
    Act = mybir.ActivationFunctionType
    Cmax = max(Cs)

    nc = bass.Bass()
    dt_in = {}

    def din(name, shape, dtype=f32):
        dt_in[name] = nc.dram_tensor(name, list(shape), dtype, kind="ExternalInput")
        return dt_in[name]

    d_times = din("times_r", [NSLOTS * CMAXG, CH])
    d_vals = din("values_r", [NSLOTS * CMAXG, CH])
    d_meas = din("meas_r", [NSLOTS * CMAXG, CH])
    d_demo = din("demo_r", [NSLOTS, 8], bf16)
    din("lens_bcast", [128, NSLOTS])
    din("cpf", [128, NF])
    din("cpb", [128, NB], bf16)
    d_out = nc.dram_tensor("out", [NSLOTS, 1], f32, kind="ExternalOutput")

    from contextlib import ExitStack
    with tile_mod.TileContext(nc) as tc, ExitStack() as stack:
        cp = stack.enter_context(tc.tile_pool(name="const", bufs=1))
        sp = stack.enter_context(tc.tile_pool(name="sbuf", bufs=1))
        pp = stack.enter_context(tc.tile_pool(name="psum", bufs=1, space="PSUM"))

        def ctile(shape, dtype=f32, name="ct"):
            return cp.tile(shape, dtype, tag=name, name=name)

        # shared PSUM bank tiles (bank-granular allocator: pack manually)
        #  mA f32: 0..263 pre (2x132) | 264..265 seg
        #  mC f32: 0..263 hh (2x132) | 264..455 tio (6x32) | 456..459 vx |
        #          460..463 vagg | 464..465 ps_a | 466..473 cr | 474..481 r3
        #  xtb bf16: 0..1015 xpose slots (2x512) | 1016..1023 ftr (2x4)
        #  "mlp" tag [128,1024] f32 x2 bufs for all MLP layers
        mA = pp.tile([128, 512], f32, tag="mA", name="mA")
        mC = pp.tile([128, 512], f32, tag="mC", name="mC")
        xtb = pp.tile([128, 1024], bf16, tag="xtb", name="xtb")

        # ---- PE warm-up (HAM clock gate) while DMAs land ----
        warm = sp.tile([128, 512], bf16, tag="warm", name="warm")
        nc.gpsimd.memset(warm[:], 0.0)
        for i in range(6):
            wps = pp.tile([128, 1024], f32, tag="mlp", bufs=2, name="wps")
            nc.tensor.matmul(wps[:, 0:512], warm[:, 0:128], warm[:, 0:512])
        # ---- packed const loads: featurize head first, rest after inputs ----
        cpf = ctile([128, NF], name="cpf")
        nc.sync.dma_start(out=cpf[:, 0:HEADF], in_=dt_in["cpf"][:, 0:HEADF])
        cpb = ctile([128, NB], bf16, name="cpb")
        nc.sync.dma_start(out=cpb[:, 0:HEADB], in_=dt_in["cpb"][:, 0:HEADB])
        lensb = ctile([128, NSLOTS], name="lensb")
        nc.sync.dma_start(out=lensb[:], in_=dt_in["lens_bcast"][:])
        demoT = ctile([8, NSLOTS], bf16, name="demoT")
        nc.sync.dma_start(out=demoT[:], in_=d_demo[:].rearrange("r f -> f r"))
        # inputs, chunk-major contiguous: 3 rows per [96,128] tile so the
        # per-row transpose stationary starts at partition 0/32/64
        tm, vl, ms = [], [], []
        for k in range(3):
            r0, r1 = k * 96, min(256, (k + 1) * 96)
            t = ctile([96, CH], name=f"tm{k}")
            nc.sync.dma_start(out=t[0:r1 - r0, :], in_=d_times[r0:r1, :])
            tm.append(t)
            t = ctile([96, CH], name=f"vl{k}")
            nc.sync.dma_start(out=t[0:r1 - r0, :], in_=d_vals[r0:r1, :])
            vl.append(t)
            t = ctile([96, CH], name=f"ms{k}")
            nc.gpsimd.dma_start(out=t[0:r1 - r0, :], in_=d_meas[r0:r1, :])
            ms.append(t)
        nc.sync.dma_start(out=cpf[:, HEADF:NF], in_=dt_in["cpf"][:, HEADF:NF])
        nc.sync.dma_start(out=cpb[:, HEADB:NB], in_=dt_in["cpb"][:, HEADB:NB])

        identf = cpf[:, F_IDENTF:F_IDENTF + 128]
        id32t = cpf[:, F_ID32:F_ID32 + 32]
        ident = cpb[:, G_IDENT:G_IDENT + 128]
        itp8 = cpf[:, F_ITP8:F_ITP8 + 8]
        iota15 = cpf[:, F_IOTA15:F_IOTA15 + NMOD]
        iotatok = cpf[:, F_IOTATOK:F_IOTATOK + 33]
        wqt = cpf[0:DP, F_WQT:F_WQT + 4]
        wkx = cpf[0:PHI_IN, F_WKX:F_WKX + 256]
        wkagg = cpf[:, F_WKAGG:F_WKAGG + 256]
        w_phi1e = cpb[0:33, G_PW1E:G_PW1E + 128]
        w_phi2 = cpb[:, G_PW2:G_PW2 + 128]
        w_phi3 = cpb[:, G_PW3:G_PW3 + 128]
        w_psi1e = cpb[0:33, G_SW1E:G_SW1E + 128]
        w_psi2 = cpb[:, G_SW2:G_SW2 + 128]
        w_psi3 = cpb[:, G_SW3:G_SW3 + 128]
        w_demo1 = cpb[0:8, G_DW1:G_DW1 + 128]
        w_demo2 = cpb[:, G_DW2:G_DW2 + 32]
        w_rattn = cpf[:, F_RATTW:F_RATTW + 128]
        w_rho1 = cpb[:, G_RW1:G_RW1 + 512].rearrange("p (h m) -> p h m", h=4)
        w_rho2 = cpf[:, F_RW2:F_RW2 + 128]
        w_rho3 = cpf[:, F_RW3:F_RW3 + 1]
        b_demo1 = cpf[:, F_BD1:F_BD1 + 1]
        b_demo2 = cpf[0:32, F_BD2:F_BD2 + 1]
        b_phi2 = cpf[:, F_BP2:F_BP2 + 1]
        b_phi3 = cpf[:, F_BP3:F_BP3 + 1]
        b_psi2 = cpf[:, F_BS2:F_BS2 + 1]
        b_psi3 = cpf[:, F_BS3:F_BS3 + 1]
        b_rattn = cpf[:, F_BRA:F_BRA + 1]
        b_rho1 = cpf[:, F_BR1:F_BR1 + 1]
        b_rho2 = cpf[:, F_BR2:F_BR2 + 1]
        nb_rho3 = cpf[0:1, F_BNR3:F_BNR3 + 1]

        # ---- derived small tensors ----
        lp1 = ctile([128, NSLOTS], name="lp1")
        nc.vector.tensor_scalar(lp1[:], lensb[:], 1.0, None, Alu.add)
        recipL1 = ctile([128, NSLOTS], name="recipL1")
        nc.vector.reciprocal(recipL1[:], lp1[:])

        # Vx [32,4] bf16, Vagg [128,4] f32 (folded W_k @ W_q per head)
        ps_vx = mC[:, 456:460]
        ps_vagg = mC[:, 460:464]
        Vx = ctile([PHI_IN, H], bf16, name="Vx")
        Vagg = ctile([128, H], name="Vagg")
        for h in range(H):
            pxt = pp.tile([128, 1024], f32, tag="mlp", bufs=2, name="pxt")
            nc.tensor.transpose(pxt[0:DP, 0:PHI_IN], wkx[:, h * DP:(h + 1) * DP],
                                identf[0:PHI_IN, 0:PHI_IN])
            sxt = sp.tile([DP, 128], f32, tag="sxt", bufs=2, name="sxt")
            nc.vector.tensor_copy(sxt[:, 0:PHI_IN], pxt[0:DP, 0:PHI_IN])
            nc.tensor.matmul(ps_vx[0:PHI_IN, h:h + 1], sxt[:, 0:PHI_IN], wqt[:, h:h + 1])
            pxt2 = pp.tile([128, 1024], f32, tag="mlp", bufs=2, name="pxt2")
            nc.tensor.transpose(pxt2[0:DP, 0:128], wkagg[:, h * DP:(h + 1) * DP], identf[:, :])
            sxt2 = sp.tile([DP, 128], f32, tag="sxt", bufs=2, name="sxt2")
            nc.vector.tensor_copy(sxt2[:], pxt2[0:DP, 0:128])
            nc.tensor.matmul(ps_vagg[:, h:h + 1], sxt2[:], wqt[:, h:h + 1])
        nc.vector.tensor_copy(Vx[:], ps_vx[0:PHI_IN, :])
        nc.vector.tensor_copy(Vagg[:], ps_vagg[:])

        # ---- demo encoder for all 8 slots ----
        ps_d = pp.tile([128, 1024], f32, tag="mlp", bufs=2, name="ps_d")
        nc.tensor.matmul(ps_d[:, 0:NSLOTS], w_demo1, demoT[:])
        dh1 = ctile([128, NSLOTS], bf16, name="dh1")
        nc.scalar.activation(dh1[:], ps_d[:, 0:NSLOTS], Act.Relu, bias=b_demo1)
        ps_d2 = pp.tile([128, 1024], f32, tag="mlp", bufs=2, name="ps_d2")
        nc.tensor.matmul(ps_d2[0:PHI_IN, 0:NSLOTS], w_demo2, dh1[:])
        demo_encT = ctile([PHI_IN + 1, NSLOTS], bf16, name="demo_encT")
        nc.scalar.activation(demo_encT[0:PHI_IN, :], ps_d2[0:PHI_IN, 0:NSLOTS],
                             Act.Identity, bias=b_demo2)
        nc.gpsimd.memset(demo_encT[PHI_IN:PHI_IN + 1, :], 1.0)

        def mlp3(rhs, ncols, out_dtype, w1e, w2, b2, w3, b3, pre):
            cur = rhs
            for li, (w, b, din_) in enumerate([
                    (w1e, None, PHI_IN + 1), (w2, b2, 128), (w3, b3, 128)]):
                ps = pp.tile([128, 1024], f32, tag="mlp", bufs=2, name=f"ps_{pre}{li}")
                nc.tensor.matmul(ps[:, 0:ncols], w, cur[0:din_, 0:ncols])
                dt_ = out_dtype if li == 2 else bf16
                nxt = sp.tile([128, NSLOTS], dt_, tag=f"demo_{pre}{li}", name=f"dm_{pre}{li}")
                nc.scalar.activation(nxt[:, 0:ncols], ps[:, 0:ncols], Act.Relu,
                                     bias=b if b is not None else 0.0)
                cur = nxt
            return cur

        denc_fm = mlp3(demo_encT, NSLOTS, bf16, w_phi1e, w_phi2, b_phi2, w_phi3, b_phi3, "phi")
        psi_demo = mlp3(demo_encT, NSLOTS, f32, w_psi1e, w_psi2, b_psi2, w_psi3, b_psi3, "psi")
        nc.tensor.transpose(xtb[0:NSLOTS, 0:128], denc_fm[:, 0:NSLOTS], ident)
        enc_demo_tok = ctile([NSLOTS, 128], bf16, name="enc_demo_tok")
        nc.vector.tensor_copy(enc_demo_tok[:], xtb[0:NSLOTS, 0:128])

        feat_all = sp.tile([128, NSLOTS, H], bf16, tag="feat_all", name="feat_all")

        rot = [0]

        def evac_relu(out_ap, in_ap, bias):
            e = rot[0] % 2
            rot[0] += 1
            if e == 0:
                nc.scalar.activation(out_ap, in_ap, Act.Relu,
                                     bias=bias if bias is not None else 0.0)
            elif bias is None:
                nc.vector.tensor_scalar(out_ap, in_ap, 0.0, None, Alu.max)
            else:
                nc.vector.tensor_scalar(out_ap, in_ap, bias, 0.0, Alu.add, Alu.max)

        def evac_copy(out_ap, in_ap):
            e = rot[0] % 2
            rot[0] += 1
            if e == 0:
                nc.scalar.copy(out_ap, in_ap)
            else:
                nc.vector.tensor_copy(out_ap, in_ap)

        xTs, masks, encs, vxes = [], [], [], []

        # ================= pre-pass (featurize + transpose) =================
        def prepass(r):
            C = Cs[r]
            Tp = C * CH
            tsrc = tm[r // 3]
            vsrc = vl[r // 3]
            msrc = ms[r // 3]
            off = (r % 3) * 32
            tb = 264 + (r % 2) * 96
            tio_t = mC[:, tb:tb + 32]
            nc.tensor.transpose(tio_t[:, 0:C], tsrc[off:off + C, :], id32t[off:off + C, 0:C])
            tio_v = mC[:, tb + 32:tb + 64]
            nc.tensor.transpose(tio_v[:, 0:C], vsrc[off:off + C, :], id32t[off:off + C, 0:C])
            tio_m = mC[:, tb + 64:tb + 96]
            nc.tensor.transpose(tio_m[:, 0:C], msrc[off:off + C, :], id32t[off:off + C, 0:C])

            # turns; i32 truncation (ang>=0) -> dd in [0,1); fold via
            # (dd>0.5)-dd and Sin(scale=-2pi)
            ang8 = sp.tile([128, Cmax, 8], f32, tag="ang", bufs=2, name="ang8")
            nc.vector.tensor_tensor(
                out=ang8[:, 0:C, :],
                in0=tio_t[:, 0:C].unsqueeze(2).to_broadcast([128, C, 8]),
                in1=itp8.unsqueeze(1).to_broadcast([128, C, 8]),
                op=Alu.mult)
            rnd = sp.tile([128, Cmax, 8], i32, tag="rnd", bufs=2, name="rnd")
            nc.vector.tensor_copy(rnd[:, 0:C, :], ang8[:, 0:C, :])
            rndf = sp.tile([128, Cmax, 8], f32, tag="rndf", bufs=2, name="rndf")
            nc.scalar.copy(rndf[:, 0:C, :], rnd[:, 0:C, :])
            dd = sp.tile([128, Cmax, 8], f32, tag="dd", bufs=2, name="dd")
            nc.vector.tensor_tensor(out=dd[:, 0:C, :], in0=ang8[:, 0:C, :],
                                    in1=rndf[:, 0:C, :], op=Alu.subtract)
            rsin = sp.tile([128, Cmax, 8], f32, tag="rsin", bufs=2, name="rsin")
            nc.vector.scalar_tensor_tensor(
                out=rsin[:, 0:C, :], in0=dd[:, 0:C, :], scalar=0.5,
                in1=dd[:, 0:C, :], op0=Alu.is_gt, op1=Alu.subtract)
            d2 = sp.tile([128, Cmax, 8], f32, tag="d2", bufs=2, name="d2")
            nc.vector.tensor_scalar(d2[:, 0:C, :], dd[:, 0:C, :], 0.25, None, Alu.add)
            rcos = sp.tile([128, Cmax, 8], f32, tag="rcos", bufs=2, name="rcos")
            nc.vector.scalar_tensor_tensor(
                out=rcos[:, 0:C, :], in0=d2[:, 0:C, :], scalar=0.5,
                in1=d2[:, 0:C, :], op0=Alu.is_gt, op1=Alu.subtract)

            xtok = sp.tile([128, Cmax, 33], bf16, tag="xtok", bufs=2, name="xtok")
            nc.scalar.activation(xtok[:, 0:C, 0:8], rsin[:, 0:C, :], Act.Sin,
                                 scale=float(-TWOPI))
            nc.scalar.activation(xtok[:, 0:C, 8:16], rcos[:, 0:C, :], Act.Sin,
                                 scale=float(-TWOPI))
            nc.vector.tensor_copy(xtok[:, 0:C, 16:17], tio_v[:, 0:C].unsqueeze(2))
            nc.vector.tensor_tensor(
                out=xtok[:, 0:C, 17:32],
                in0=tio_m[:, 0:C].unsqueeze(2).to_broadcast([128, C, NMOD]),
                in1=iota15.unsqueeze(1).to_broadcast([128, C, NMOD]),
                op=Alu.is_equal)
            nc.gpsimd.memset(xtok[:, 0:C, 32:33], 1.0)

            xT = sp.tile([33, (C + 1) * CH], bf16, tag=f"xT{r}", name="xT")
            xTs.append(xT)
            for g in range((C + 3) // 4):
                c0 = g * 4
                nch = min(4, C - c0)
                xb = (g % 2) * 512
                pxp = xtb[0:33, xb:xb + 512]
                for j in range(nch):
                    nc.tensor.transpose(pxp[:, j * CH:(j + 1) * CH],
                                        xtok[:, c0 + j, 0:33], ident)
                evac_copy(xT[:, c0 * CH:(c0 + nch) * CH], pxp[:, 0:nch * CH])
            mask_ext = sp.tile([128, Cmax + 1], bf16, tag="maskx", bufs=NSLOTS, name="mask_ext")
            masks.append(mask_ext)
            nc.vector.tensor_scalar(mask_ext[:, 0:C + 1], iotatok[:, 0:C + 1],
                                    lensb[:, r:r + 1], None, Alu.is_lt)
            nc.gpsimd.memset(mask_ext[0:1, C:C + 1], 1.0)

        # ================= main MLP per row ================================
        def main_mlp(r):
            C = Cs[r]
            Tp = C * CH
            xT = xTs[r]
            mask_ext = masks[r]

            nc.gpsimd.memset(xT[:, Tp:Tp + CH], 0.0)
            nc.scalar.copy(xT[:, Tp:Tp + 1], demo_encT[:, r:r + 1])

            h1 = sp.tile([128, Cmax * CH], bf16, tag="h_a", bufs=2, name="h1")
            h2 = sp.tile([128, Cmax * CH], bf16, tag="h_b", bufs=2, name="h2")
            p1 = sp.tile([128, Cmax * CH], bf16, tag="h_c", bufs=2, name="p1")
            p2 = sp.tile([128, Cmax * CH], bf16, tag="h_d", bufs=2, name="p2")

            def layer_k(w, b, rhs_tile, rhs_rows, out_tile, k):
                N2 = min(1024, Tp - k * 1024)
                ps = pp.tile([128, 1024], f32, tag="mlp", bufs=2, name="ps_mlp")
                for ho in range(0, N2, 512):
                    N = min(512, N2 - ho)
                    nc.tensor.matmul(ps[:, ho:ho + N], w,
                                     rhs_tile[0:rhs_rows, k * 1024 + ho:k * 1024 + ho + N])
                evac_relu(out_tile[:, k * 1024:k * 1024 + N2], ps[:, 0:N2], b)

            NK = (Tp + 1023) // 1024
            for k in range(NK):
                layer_k(w_phi1e, None, xT, 33, h1, k)
                layer_k(w_psi1e, None, xT, 33, p1, k)
            for k in range(NK):
                layer_k(w_phi2, b_phi2, h1, 128, h2, k)
                layer_k(w_psi2, b_psi2, p1, 128, p2, k)

            enc_ext = sp.tile([128, C + 1, 129], bf16, tag=f"enc{r}", name="enc_ext")
            encs.append(enc_ext)
            seg = mA[:, 264 + (r % 2):265 + (r % 2)]
            NG8 = (C + 7) // 8
            ptoks = [None] * NG8

            def seg_mms(g):
                c0 = g * 8
                nch = min(8, C - c0)
                for j in range(nch):
                    c = c0 + j
                    nc.tensor.matmul(seg[:], ptoks[g][:, j, :], mask_ext[:, c:c + 1],
                                     start=(c == 0), stop=(c == C - 1))

            for g in range(NG8):
                c0 = g * 8
                nch = min(8, C - c0)
                l3 = pp.tile([128, 8, 128], f32, tag="mlp", bufs=2, name="l3_phi")
                for j in range(nch):
                    nc.tensor.matmul(l3[:, j, :],
                                     h2[:, (c0 + j) * CH:(c0 + j + 1) * CH], w_phi3)
                evac_relu(enc_ext[:, c0:c0 + nch, 0:128], l3[:, 0:nch, :], b_phi3)
                l3p = pp.tile([128, 8, 128], f32, tag="mlp", bufs=2, name="l3_psi")
                for j in range(nch):
                    nc.tensor.matmul(l3p[:, j, :],
                                     p2[:, (c0 + j) * CH:(c0 + j + 1) * CH], w_psi3)
                ptok = sp.tile([128, 8, 128], bf16, tag="ptok", bufs=2, name="ptok")
                ptoks[g] = ptok
                evac_relu(ptok[:, 0:nch, :], l3p[:, 0:nch, :], b_psi3)
                if g >= 1:
                    seg_mms(g - 1)
            seg_mms(NG8 - 1)
            nc.gpsimd.memset(enc_ext[:, 0:C + 1, 128:129], 1.0)
            nc.gpsimd.memset(enc_ext[:, C, 0:128], 0.0)
            nc.sync.dma_start(out=enc_ext[0:1, C, 0:128], in_=enc_demo_tok[r:r + 1, :])

            agg_in = sp.tile([128, 1], f32, tag="aggin", bufs=2, name="agg_in")
            nc.vector.tensor_scalar(agg_in[:], seg[:], psi_demo[:, r:r + 1],
                                    recipL1[:, r:r + 1], Alu.add, Alu.mult)
            ps_a = mC[:, 464 + (r % 2):465 + (r % 2)]
            nc.tensor.matmul(ps_a[:], w_rattn, agg_in[:])
            agg2 = sp.tile([128, 1], f32, tag="agg2", bufs=2, name="agg2")
            nc.scalar.activation(agg2[:], ps_a[:], Act.Relu, bias=b_rattn)
            ps_cr = mC[0:1, 466 + (r % 2) * 4:470 + (r % 2) * 4]
            nc.tensor.matmul(ps_cr[:], agg2[:], Vagg[:])
            vxe = sp.tile([PHI_IN + 1, H], bf16, tag="vxe", bufs=NSLOTS, name="vxe")
            vxes.append(vxe)
            nc.vector.tensor_copy(vxe[0:PHI_IN, :], Vx[:])
            nc.vector.tensor_copy(vxe[PHI_IN:PHI_IN + 1, :], ps_cr[:])

        # ================= attention tails (hh(r) zipped with pre(r+1)) ====
        def pre_mms(r):
            C = Cs[r]
            pb = (r % 2) * 132
            pre = mA[:, pb:pb + 132]
            for c in range(C + 1):
                nc.tensor.matmul(pre[:, c * 4:c * 4 + 4],
                                 xTs[r][:, c * CH:(c + 1) * CH], vxes[r][:])
            return pre

        def exp_mask(r, pre):
            C = Cs[r]
            e_tok = sp.tile([128, (Cmax + 1) * 4], bf16, tag="etok", bufs=2, name="e_tok")
            nc.scalar.activation(e_tok[:, 0:(C + 1) * 4], pre[:, 0:(C + 1) * 4], Act.Exp)
            e_m = sp.tile([128, Cmax + 1, 4], bf16, tag="em", bufs=2, name="e_m")
            nc.vector.tensor_tensor(
                out=e_m[:, 0:C + 1, :],
                in0=e_tok[:, 0:(C + 1) * 4].rearrange("p (c h) -> p c h", h=4),
                in1=masks[r][:, 0:C + 1].unsqueeze(2).to_broadcast([128, C + 1, 4]),
                op=Alu.mult)
            return e_m

        def attn_tails():
            pre = pre_mms(0)
            for r in range(NSLOTS):
                C = Cs[r]
                e_m = exp_mask(r, pre)
                hb = (r % 2) * 132
                hh = mC[0:4, hb:hb + 132]
                # zipper: hh(r) chunk streams hide pre(r+1) weight loads
                Cn = Cs[r + 1] if r + 1 < NSLOTS else -1
                if r + 1 < NSLOTS:
                    pn = ((r + 1) % 2) * 132
                    pre = mA[:, pn:pn + 132]
                for c in range(max(C, Cn) + 1):
                    if c <= C:
                        nc.tensor.matmul(hh[:, 0:129], e_m[:, c, :], encs[r][:, c, :],
                                         start=(c == 0), stop=(c == C))
                    if c <= Cn:
                        nc.tensor.matmul(pre[:, c * 4:c * 4 + 4],
                                         xTs[r + 1][:, c * CH:(c + 1) * CH], vxes[r + 1][:])
                rz = sp.tile([4, 1], f32, tag="rz", bufs=2, name="rz")
                nc.vector.reciprocal(rz[:], hh[:, 128:129])
                hh_sb = sp.tile([4, 128], bf16, tag="hhsb", bufs=2, name="hh_sb")
                nc.vector.tensor_scalar(hh_sb[:], hh[:, 0:128], rz[:], None, Alu.mult)
                ps_tr = xtb[:, 1016 + (r % 2) * 4:1020 + (r % 2) * 4]
                nc.tensor.transpose(ps_tr[:, :], hh_sb[:], ident[0:H, 0:H])
                nc.vector.tensor_copy(feat_all[:, r, :], ps_tr[:, :])

        # ================= emission schedule ===============================
        prepass(0)
        prepass(1)
        for r in range(NSLOTS):
            if r + 2 < NSLOTS:
                prepass(r + 2)
            main_mlp(r)
        attn_tails()

        # ================= rho MLP over all 8 rows =========================
        ps_r1 = pp.tile([128, 1024], f32, tag="mlp", bufs=2, name="ps_r1")
        for h in range(H):
            nc.tensor.matmul(ps_r1[:, 0:NSLOTS], w_rho1[:, h, :], feat_all[:, :, h],
                             start=(h == 0), stop=(h == H - 1))
        r1 = sp.tile([128, NSLOTS], f32, tag="r1", name="r1")
        nc.scalar.activation(r1[:], ps_r1[:, 0:NSLOTS], Act.Relu, bias=b_rho1)
        ps_r2 = pp.tile([128, 1024], f32, tag="mlp", bufs=2, name="ps_r2")
        nc.tensor.matmul(ps_r2[:, 0:NSLOTS], w_rho2, r1[:])
        r2 = sp.tile([128, NSLOTS], f32, tag="r2", name="r2")
        nc.scalar.activation(r2[:], ps_r2[:, 0:NSLOTS], Act.Relu, bias=b_rho2)
        ps_r3 = mC[0:1, 474:474 + NSLOTS]
        nc.tensor.matmul(ps_r3[:], w_rho3, r2[:])
        # sigmoid(z) = 1/(1+exp(-z-b3)) stays within the exp table set
        en = sp.tile([1, NSLOTS], f32, tag="en", name="en")
        nc.scalar.activation(en[:], ps_r3[:], Act.Exp, bias=nb_rho3, scale=-1.0)
        ep1 = sp.tile([1, NSLOTS], f32, tag="ep1", name="ep1")
        nc.vector.tensor_scalar(ep1[:], en[:], 1.0, None, Alu.add)
        res = sp.tile([1, NSLOTS], f32, tag="res", name="res")
        nc.vector.reciprocal(res[:], ep1[:])
        nc.sync.dma_start(out=d_out[:].rearrange("r one -> one r"), in_=res[:])
    return nc


def _patch_tile_drain(tile_mod, mybir):
    """Walrus in this env rejects >1 sync wait per instruction."""
    from concourse.vector_clock import ScopedClock
    if getattr(tile_mod.TileContext, "_drain_patched", False):
        return

    def _drain_and_barrier(self, tick_clock, wait_clock):
        nc = self.nc
        drain_inst = nc.sync.drain()
        wait_clock.add_sem_waits(drain_inst.ins, ScopedClock({None: tick_clock.global_clock}))
        si = drain_inst.ins.sync_info
        waits = list(si.on_wait or [])
        if len(waits) > 1:
            si.on_wait = waits[:1]
            for i in range(1, len(waits)):
                extra = nc.sync.drain()
                esi = extra.ins.sync_info
                if esi is None:
                    extra.ins.sync_info = mybir.SyncInfo(on_wait=waits[i:i + 1], on_update=[])
                else:
                    esi.on_wait = waits[i:i + 1]
        nc.all_engine_barrier()
        popped = nc._tile_sem_poison_stack.pop()
        assert popped is self._sem_poison
        nc.clear_and_free_semaphores(list(self.sems.allocated().values()))
        nc.all_engine_barrier()

    tile_mod.TileContext._drain_and_barrier = _drain_and_barrier

    _orig_exit = tile_mod.TileContext.__exit__

    def _exit(self, exc_type, exc_val, exc_tb):
        r = _orig_exit(self, exc_type, exc_val, exc_tb)
        if exc_type is None and getattr(tile_mod.TileContext, "_split_waits", True):
            _split_multi_waits(self.nc, mybir)
        return r

    def _split_multi_waits(nc, mybir):
        n = [0]
        for f in nc.m.functions:
            for bb in f.blocks:
                insts = bb.instructions
                out = []
                for inst in insts:
                    si = inst.sync_info
                    waits = list(si.on_wait) if (si and si.on_wait) else []
                    if len(waits) > 1:
                        for w in waits[:-1]:
                            n[0] += 1
                            nop = mybir.InstNoOp(name=f"I-ws-{n[0]}", ins=[], outs=[])
                            nop.engine = inst.engine
                            nop.sync_info = mybir.SyncInfo(on_wait=[w], on_update=[])
                            out.append(nop)
                        si.on_wait = waits[-1:]
                    out.append(inst)
                if len(out) != len(insts):
                    bb.instructions = out

    tile_mod.TileContext.__exit__ = _exit
    tile_mod.TileContext._drain_patched = True


_CACHE = {}
last_results = None


def _maybe_install_ntff_shim():
    import sys, types
    if "antenv.axon_hooks" in sys.modules:
        return
    try:
        from trn_agent_boot.trn_boot import _ntff_profile_via_ctypes
        hook = _ntff_profile_via_ctypes("/opt/axon/libaxon_pjrt.so")
    except Exception:
        hook = None
    mod = types.ModuleType("antenv.axon_hooks")
    mod.get_axon_ntff_profile_hook = lambda: hook
    sys.modules["antenv.axon_hooks"] = mod


def _to_bf16(a):
    import ml_dtypes
    return np.asarray(a, np.float32).astype(ml_dtypes.bfloat16)


def _build_packs(inp):
    """Host-side packed constant tensors (one f32, one bf16)."""
    f = np.zeros((128, NF), np.float32)
    f[:, F_ID32:F_ID32 + 32] = np.tile(np.eye(32, dtype=np.float32), (4, 1))
    f[:, F_IDENTF:F_IDENTF + 128] = np.eye(128, dtype=np.float32)
    f[:, F_RATTW:F_RATTW + 128] = inp["rho_attn_w"]
    f[:, F_RW2:F_RW2 + 128] = inp["rho_w2"]
    f[:, F_RW3:F_RW3 + 1] = inp["rho_w3"]
    f[:, F_WKAGG:F_WKAGG + 256] = inp["W_k"][PHI_IN:]
    f[0:PHI_IN, F_WKX:F_WKX + 256] = inp["W_k"][0:PHI_IN]
    wq = inp["W_q"].astype(np.float32) * (1.0 / np.sqrt(DP))
    f[0:DP, F_WQT:F_WQT + 4] = wq.T
    ts = MAXTS ** np.linspace(0.0, 1.0, NPOS // 2).astype(np.float32)
    f[:, F_ITP8:F_ITP8 + 8] = (1.0 / (ts * TWOPI))[None, :]
    f[:, F_IOTA15:F_IOTA15 + NMOD] = np.arange(NMOD, dtype=np.float32)[None, :]
    f[:, F_IOTATOK:F_IOTATOK + 33] = (np.arange(128, dtype=np.float32)[:, None]
                                      + 128.0 * np.arange(33, dtype=np.float32)[None, :])
    f[:, F_BD1] = inp["demo_b1"]
    f[:, F_BP2] = inp["phi_b2"]
    f[:, F_BP3] = inp["phi_b3"]
    f[:, F_BS2] = inp["psi_b2"]
    f[:, F_BS3] = inp["psi_b3"]
    f[:, F_BRA] = inp["rho_attn_b"]
    f[:, F_BR1] = inp["rho_b1"]
    f[:, F_BR2] = inp["rho_b2"]
    f[0:32, F_BD2] = inp["demo_b2"]
    f[0, F_BNR3] = -inp["rho_b3"][0]

    g = np.zeros((128, NB), np.float32)
    g[:, G_IDENT:G_IDENT + 128] = np.eye(128, dtype=np.float32)
    g[:, G_PW2:G_PW2 + 128] = inp["phi_w2"]
    g[:, G_PW3:G_PW3 + 128] = inp["phi_w3"]
    g[:, G_SW2:G_SW2 + 128] = inp["psi_w2"]
    g[:, G_SW3:G_SW3 + 128] = inp["psi_w3"]
    g[:, G_RW1:G_RW1 + 512] = inp["rho_w1"].reshape(4, 128, 128).transpose(1, 0, 2).reshape(128, 512)
    g[0:33, G_PW1E:G_PW1E + 128] = np.concatenate(
        [inp["phi_w1"], inp["phi_b1"][None, :]], axis=0)
    g[0:33, G_SW1E:G_SW1E + 128] = np.concatenate(
        [inp["psi_w1"], inp["psi_b1"][None, :]], axis=0)
    g[0:8, G_DW1:G_DW1 + 128] = inp["demo_w1"]
    g[:, G_DW2:G_DW2 + 32] = inp["demo_w2"]
    return f, _to_bf16(g)


def _prepare(inputs):
    import concourse.bass as bass
    import concourse.mybir as mybir
    import concourse.tile as tile_mod

    _patch_tile_drain(tile_mod, mybir)

    inp = {k: np.asarray(v, np.float32) if np.asarray(v).dtype != np.float32 else np.asarray(v)
           for k, v in inputs.items()}
    times = np.ascontiguousarray(inp["times"][..., 0])    # [B,T]
    values = np.ascontiguousarray(inp["values"][..., 0])  # [B,T]
    meas = np.ascontiguousarray(inputs["measurements"]).astype(np.float32)
    lengths = np.asarray(inputs["lengths"]).astype(np.int64)
    demo = inp["demo"]

    order = np.argsort(-lengths, kind="stable")
    Cs = []
    for s in range(NSLOTS):
        ranks = order[s * NCORES:(s + 1) * NCORES]
        Cs.append(int(np.ceil(lengths[ranks].max() / CH)))

    key = tuple(Cs)
    if key not in _CACHE:
        _CACHE[key] = _build_nc(Cs, tile_mod, bass, mybir)
    nc = _CACHE[key]

    cpf, cpb = _build_packs(inp)

    in_maps = []
    for core in range(NCORES):
        rows = [order[s * NCORES + core] for s in range(NSLOTS)]
        m = {
            "times_r": times[rows].reshape(NSLOTS * CMAXG, CH),
            "values_r": values[rows].reshape(NSLOTS * CMAXG, CH),
            "meas_r": meas[rows].reshape(NSLOTS * CMAXG, CH),
            "demo_r": _to_bf16(demo[rows]),
            "lens_bcast": np.ascontiguousarray(
                np.broadcast_to(lengths[rows].astype(np.float32)[None, :], (128, NSLOTS))),
            "cpf": cpf, "cpb": cpb,
        }
        in_maps.append(m)
    return nc, in_maps, order


def kernel(**inputs):
    import os
    from concourse import bass_utils

    nc, in_maps, order = _prepare(inputs)

    trace = os.environ.get("KERNEL_TRACE", "0") == "1"
    kw = {}
    if trace:
        _maybe_install_ntff_shim()
        kw = dict(trace=True, tmpdir=os.environ.get("KERNEL_TRACE_DIR") or None)
    res = bass_utils.run_bass_kernel_spmd(nc, in_maps, core_ids=list(range(NCORES)), **kw)
    global last_results
    last_results = res
    out = np.zeros((B, 1), np.float32)
    for core in range(NCORES):
        for s in range(NSLOTS):
            out[order[s * NCORES + core], 0] = res.results[core]["out"][s, 0]
    return out


# revision 9
# speedup vs baseline: 1.0527x; 1.0527x over previous
"""Trainium2 Bass kernel for nn_DeepSetAttentionModel (segment_reduce) — v3.

v3 vs v2:
- All constants/weights packed host-side into TWO big DMAs (one f32
  pack, one bf16 pack) instead of ~45 small ones (the v2 trace showed a
  60us DMA-serialized pre-pass on the sync queue).
- Inputs loaded contiguously chunk-major ([128,128] tiles, 2 DMAs per
  tensor) and PE-transposed to token-major (strided 4B-gather DMAs were
  ~1.4us each).
- Emission interleaved: featurize runs 2 rows ahead of the MLP loop;
  attention tails (which contain Exp) run after all Sins -> still only
  2 act-table loads.
- Featurize fused with scalar_tensor_tensor + Sin(scale=-2pi) fold.
Everything else (token-major attention, chunk-stationary L3, ones-col
Z, rotating Act/DVE evacuations) as in v2.
"""
import numpy as np

B, T = 64, 4096
CH = 128
NPOS, V, NMOD = 16, 1, 15
PHI_IN = 32
DP, H = 64, 4
MAXTS = 100.0
NCORES = 8
NSLOTS = B // NCORES
TWOPI = 2.0 * np.pi
CMAXG = 32          # chunks per row in the packed input layout

# f32 pack column offsets (head = first HEADF cols, DMA'd first)
F_ID32, F_ITP8, F_IOTA15, F_IOTATOK = 0, 32, 40, 55
F_BD1, F_BP2, F_BP3, F_BS2, F_BS3, F_BRA, F_BR1, F_BR2 = 88, 89, 90, 91, 92, 93, 94, 95
F_BD2, F_BNR3, F_VAGG = 96, 97, 98
HEADF = 102
F_RATTW, F_RW2, F_RW3 = 102, 230, 358
NF = 359
# bf16 pack: H1 = ident + layer-1 weights (+demo, Vx); H2 = w2/w3; rho_w1 last
G_IDENT, G_W1A, G_W1B, G_DW2, G_VX = 0, 128, 256, 384, 416
HEADB = 420
G_PW2, G_SW2, G_PW3, G_SW3 = 420, 548, 676, 804
HEAD2B = 932
G_RW1 = 932
NB = 1444


def _build_nc(Cs, tile_mod, bass, mybir):
    f32 = mybir.dt.float32
    bf16 = mybir.dt.bfloat16
    i32 = mybir.dt.int32
    Alu = mybir.AluOpType
    Act = mybir.ActivationFunctionType
    Cmax = max(Cs)

    nc = bass.Bass()
    dt_in = {}

    def din(name, shape, dtype=f32):
        dt_in[name] = nc.dram_tensor(name, list(shape), dtype, kind="ExternalInput")
        return dt_in[name]

    d_times = din("times_r", [NSLOTS * CMAXG, CH])
    d_vals = din("values_r", [NSLOTS * CMAXG, CH])
    d_meas = din("meas_r", [NSLOTS * CMAXG, CH])
    d_demo = din("demo_r", [NSLOTS, 8], bf16)
    din("lens_bcast", [128, NSLOTS])
    din("cpf", [128, NF])
    din("cpb", [128, NB], bf16)
    d_out = nc.dram_tensor("out", [NSLOTS, 1], f32, kind="ExternalOutput")

    from contextlib import ExitStack
    with tile_mod.TileContext(nc) as tc, ExitStack() as stack:
        cp = stack.enter_context(tc.tile_pool(name="const", bufs=1))
        sp = stack.enter_context(tc.tile_pool(name="sbuf", bufs=1))
        pp = stack.enter_context(tc.tile_pool(name="psum", bufs=1, space="PSUM"))

        def ctile(shape, dtype=f32, name="ct"):
            return cp.tile(shape, dtype, tag=name, name=name)

        # shared PSUM bank tiles (bank-granular allocator: pack manually)
        #  mA f32: 0..263 pre (2x132) | 264..265 seg
        #  mC f32: 0..263 hh (2x132) | 264..455 tio (6x32) | 456..459 vx |
        #          460..463 vagg | 464..465 ps_a | 466..473 cr | 474..481 r3
        #  xtb bf16: 0..1015 xpose slots (2x512) | 1016..1023 ftr (2x4)
        #  "mlp" tag [128,1024] f32 x2 bufs for all MLP layers
        mA = pp.tile([128, 512], f32, tag="mA", name="mA")
        mC = pp.tile([128, 512], f32, tag="mC", name="mC")
        mS = pp.tile([128, 16], f32, tag="mS", name="mS")
        xtb = pp.tile([128, 1024], bf16, tag="xtb", name="xtb")

        # ---- PE warm-up (HAM clock gate) while DMAs land ----
        warm = sp.tile([128, 512], bf16, tag="warm", name="warm")
        nc.gpsimd.memset(warm[:], 0.0)
        for i in range(6):
            wps = pp.tile([128, 1024], f32, tag="mlp", bufs=2, name="wps")
            nc.tensor.matmul(wps[:, 0:512], warm[:, 0:128], warm[:, 0:512])
        # ---- packed const loads, spread across engine DMA queues ----
        cpf = ctile([128, NF], name="cpf")
        nc.sync.dma_start(out=cpf[:, 0:HEADF], in_=dt_in["cpf"][:, 0:HEADF])
        cpb = ctile([128, NB], bf16, name="cpb")
        nc.scalar.dma_start(out=cpb[:, 0:HEADB], in_=dt_in["cpb"][:, 0:HEADB])
        lensb = ctile([128, NSLOTS], name="lensb")
        nc.sync.dma_start(out=lensb[:], in_=dt_in["lens_bcast"][:])
        demoT = ctile([72, NSLOTS], bf16, name="demoT")
        nc.sync.dma_start(out=demoT[64:72, :], in_=d_demo[:].rearrange("r f -> f r"))
        # inputs, chunk-major contiguous: 3 rows per [96,128] tile so the
        # per-row transpose stationary starts at partition 0/32/64
        tm, vl, ms = [], [], []
        for k in range(3):
            r0, r1 = k * 96, min(256, (k + 1) * 96)
            t = ctile([96, CH], name=f"tm{k}")
            nc.sync.dma_start(out=t[0:r1 - r0, :], in_=d_times[r0:r1, :])
            tm.append(t)
            t = ctile([96, CH], name=f"vl{k}")
            nc.scalar.dma_start(out=t[0:r1 - r0, :], in_=d_vals[r0:r1, :])
            vl.append(t)
            t = ctile([96, CH], name=f"ms{k}")
            nc.gpsimd.dma_start(out=t[0:r1 - r0, :], in_=d_meas[r0:r1, :])
            ms.append(t)
        nc.scalar.dma_start(out=cpb[:, HEADB:HEAD2B], in_=dt_in["cpb"][:, HEADB:HEAD2B])
        nc.sync.dma_start(out=cpf[:, HEADF:NF], in_=dt_in["cpf"][:, HEADF:NF])
        nc.scalar.dma_start(out=cpb[:, HEAD2B:NB], in_=dt_in["cpb"][:, HEAD2B:NB])

        id32t = cpf[:, F_ID32:F_ID32 + 32]
        ident = cpb[:, G_IDENT:G_IDENT + 128]
        itp8 = cpf[:, F_ITP8:F_ITP8 + 8]
        iota15 = cpf[:, F_IOTA15:F_IOTA15 + NMOD]
        iotatok = cpf[:, F_IOTATOK:F_IOTATOK + 33]
        w_phi1e = cpb[0:33, G_W1A:G_W1A + 128]
        w_demo1 = cpb[64:72, G_W1A:G_W1A + 128]
        w_psi1e = cpb[0:33, G_W1B:G_W1B + 128]
        w_phi2 = cpb[:, G_PW2:G_PW2 + 128]
        w_phi3 = cpb[:, G_PW3:G_PW3 + 128]
        w_psi2 = cpb[:, G_SW2:G_SW2 + 128]
        w_psi3 = cpb[:, G_SW3:G_SW3 + 128]
        w_demo2 = cpb[:, G_DW2:G_DW2 + 32]
        Vx = cpb[0:PHI_IN, G_VX:G_VX + 4]
        Vagg = cpf[:, F_VAGG:F_VAGG + 4]
        w_rattn = cpf[:, F_RATTW:F_RATTW + 128]
        w_rho1 = cpb[:, G_RW1:G_RW1 + 512].rearrange("p (h m) -> p h m", h=4)
        w_rho2 = cpf[:, F_RW2:F_RW2 + 128]
        w_rho3 = cpf[:, F_RW3:F_RW3 + 1]
        b_demo1 = cpf[:, F_BD1:F_BD1 + 1]
        b_demo2 = cpf[0:32, F_BD2:F_BD2 + 1]
        b_phi2 = cpf[:, F_BP2:F_BP2 + 1]
        b_phi3 = cpf[:, F_BP3:F_BP3 + 1]
        b_psi2 = cpf[:, F_BS2:F_BS2 + 1]
        b_psi3 = cpf[:, F_BS3:F_BS3 + 1]
        b_rattn = cpf[:, F_BRA:F_BRA + 1]
        b_rho1 = cpf[:, F_BR1:F_BR1 + 1]
        b_rho2 = cpf[:, F_BR2:F_BR2 + 1]
        nb_rho3 = cpf[0:1, F_BNR3:F_BNR3 + 1]

        # ---- derived small tensors ----
        lp1 = ctile([128, NSLOTS], name="lp1")
        nc.vector.tensor_scalar(lp1[:], lensb[:], 1.0, None, Alu.add)
        recipL1 = ctile([128, NSLOTS], name="recipL1")
        nc.vector.reciprocal(recipL1[:], lp1[:])

        # ---- demo encoder for all 8 slots ----
        ps_d = pp.tile([128, 1024], f32, tag="mlp", bufs=2, name="ps_d")
        nc.tensor.matmul(ps_d[:, 0:NSLOTS], w_demo1, demoT[64:72, :])
        dh1 = ctile([128, NSLOTS], bf16, name="dh1")
        nc.scalar.activation(dh1[:], ps_d[:, 0:NSLOTS], Act.Relu, bias=b_demo1)
        ps_d2 = pp.tile([128, 1024], f32, tag="mlp", bufs=2, name="ps_d2")
        nc.tensor.matmul(ps_d2[0:PHI_IN, 0:NSLOTS], w_demo2, dh1[:])
        demo_encT = ctile([PHI_IN + 1, NSLOTS], bf16, name="demo_encT")
        nc.scalar.activation(demo_encT[0:PHI_IN, :], ps_d2[0:PHI_IN, 0:NSLOTS],
                             Act.Identity, bias=b_demo2)
        nc.gpsimd.memset(demo_encT[PHI_IN:PHI_IN + 1, :], 1.0)

        def mlp3(rhs, ncols, out_dtype, w1e, w2, b2, w3, b3, pre):
            cur = rhs
            for li, (w, b, din_) in enumerate([
                    (w1e, None, PHI_IN + 1), (w2, b2, 128), (w3, b3, 128)]):
                ps = pp.tile([128, 1024], f32, tag="mlp", bufs=2, name=f"ps_{pre}{li}")
                nc.tensor.matmul(ps[:, 0:ncols], w, cur[0:din_, 0:ncols])
                dt_ = out_dtype if li == 2 else bf16
                nxt = sp.tile([128, NSLOTS], dt_, tag=f"demo_{pre}{li}", name=f"dm_{pre}{li}")
                nc.scalar.activation(nxt[:, 0:ncols], ps[:, 0:ncols], Act.Relu,
                                     bias=b if b is not None else 0.0)
                cur = nxt
            return cur

        denc_fm = mlp3(demo_encT, NSLOTS, bf16, w_phi1e, w_phi2, b_phi2, w_phi3, b_phi3, "phi")
        psi_demo = mlp3(demo_encT, NSLOTS, f32, w_psi1e, w_psi2, b_psi2, w_psi3, b_psi3, "psi")
        nc.tensor.transpose(xtb[0:NSLOTS, 0:128], denc_fm[:, 0:NSLOTS], ident)
        enc_demo_tok = ctile([NSLOTS, 128], bf16, name="enc_demo_tok")
        nc.vector.tensor_copy(enc_demo_tok[:], xtb[0:NSLOTS, 0:128])

        feat_all = sp.tile([128, NSLOTS, H], bf16, tag="feat_all", name="feat_all")

        rot = [0]

        def evac_relu(out_ap, in_ap, bias):
            e = rot[0] % 2
            rot[0] += 1
            if e == 0:
                nc.scalar.activation(out_ap, in_ap, Act.Relu,
                                     bias=bias if bias is not None else 0.0)
            elif bias is None:
                nc.vector.tensor_scalar(out_ap, in_ap, 0.0, None, Alu.max)
            else:
                nc.vector.tensor_scalar(out_ap, in_ap, bias, 0.0, Alu.add, Alu.max)

        def evac_copy(out_ap, in_ap):
            e = rot[0] % 2
            rot[0] += 1
            if e == 0:
                nc.scalar.copy(out_ap, in_ap)
            else:
                nc.vector.tensor_copy(out_ap, in_ap)

        xTs, masks, encs, vxes = [], [], [], []

        # ================= pre-pass (featurize + transpose) =================
        def prepass(r):
            C = Cs[r]
            Tp = C * CH
            tsrc = tm[r // 3]
            vsrc = vl[r // 3]
            msrc = ms[r // 3]
            off = (r % 3) * 32
            tb = 264 + (r % 2) * 96
            tio_t = mC[:, tb:tb + 32]
            nc.tensor.transpose(tio_t[:, 0:C], tsrc[off:off + C, :], id32t[off:off + C, 0:C])
            tio_v = mC[:, tb + 32:tb + 64]
            nc.tensor.transpose(tio_v[:, 0:C], vsrc[off:off + C, :], id32t[off:off + C, 0:C])
            tio_m = mC[:, tb + 64:tb + 96]
            nc.tensor.transpose(tio_m[:, 0:C], msrc[off:off + C, :], id32t[off:off + C, 0:C])

            # turns; i32 truncation (ang>=0) -> dd in [0,1); fold via
            # (dd>0.5)-dd and Sin(scale=-2pi)
            ang8 = sp.tile([128, Cmax, 8], f32, tag="ang", bufs=2, name="ang8")
            nc.vector.tensor_tensor(
                out=ang8[:, 0:C, :],
                in0=tio_t[:, 0:C].unsqueeze(2).to_broadcast([128, C, 8]),
                in1=itp8.unsqueeze(1).to_broadcast([128, C, 8]),
                op=Alu.mult)
            rnd = sp.tile([128, Cmax, 8], i32, tag="rnd", bufs=2, name="rnd")
            nc.vector.tensor_copy(rnd[:, 0:C, :], ang8[:, 0:C, :])
            rndf = sp.tile([128, Cmax, 8], f32, tag="rndf", bufs=2, name="rndf")
            nc.scalar.copy(rndf[:, 0:C, :], rnd[:, 0:C, :])
            dd = sp.tile([128, Cmax, 8], f32, tag="dd", bufs=2, name="dd")
            nc.vector.tensor_tensor(out=dd[:, 0:C, :], in0=ang8[:, 0:C, :],
                                    in1=rndf[:, 0:C, :], op=Alu.subtract)
            rsin = sp.tile([128, Cmax, 8], f32, tag="rsin", bufs=2, name="rsin")
            nc.vector.scalar_tensor_tensor(
                out=rsin[:, 0:C, :], in0=dd[:, 0:C, :], scalar=0.5,
                in1=dd[:, 0:C, :], op0=Alu.is_gt, op1=Alu.subtract)
            d2 = sp.tile([128, Cmax, 8], f32, tag="d2", bufs=2, name="d2")
            nc.vector.tensor_scalar(d2[:, 0:C, :], dd[:, 0:C, :], 0.25, None, Alu.add)
            rcos = sp.tile([128, Cmax, 8], f32, tag="rcos", bufs=2, name="rcos")
            nc.vector.scalar_tensor_tensor(
                out=rcos[:, 0:C, :], in0=d2[:, 0:C, :], scalar=0.5,
                in1=d2[:, 0:C, :], op0=Alu.is_gt, op1=Alu.subtract)

            xtok = sp.tile([128, Cmax, 33], bf16, tag="xtok", bufs=2, name="xtok")
            nc.scalar.activation(xtok[:, 0:C, 0:8], rsin[:, 0:C, :], Act.Sin,
                                 scale=float(-TWOPI))
            nc.scalar.activation(xtok[:, 0:C, 8:16], rcos[:, 0:C, :], Act.Sin,
                                 scale=float(-TWOPI))
            nc.vector.tensor_copy(xtok[:, 0:C, 16:17], tio_v[:, 0:C].unsqueeze(2))
            nc.vector.tensor_tensor(
                out=xtok[:, 0:C, 17:32],
                in0=tio_m[:, 0:C].unsqueeze(2).to_broadcast([128, C, NMOD]),
                in1=iota15.unsqueeze(1).to_broadcast([128, C, NMOD]),
                op=Alu.is_equal)
            nc.gpsimd.memset(xtok[:, 0:C, 32:33], 1.0)

            xT = sp.tile([33, (C + 1) * CH], bf16, tag=f"xT{r}", name="xT")
            xTs.append(xT)
            for g in range((C + 3) // 4):
                c0 = g * 4
                nch = min(4, C - c0)
                xb = (g % 2) * 512
                pxp = xtb[0:33, xb:xb + 512]
                for j in range(nch):
                    nc.tensor.transpose(pxp[:, j * CH:(j + 1) * CH],
                                        xtok[:, c0 + j, 0:33], ident)
                evac_copy(xT[:, c0 * CH:(c0 + nch) * CH], pxp[:, 0:nch * CH])
            mask_ext = sp.tile([128, Cmax + 1], bf16, tag="maskx", bufs=NSLOTS, name="mask_ext")
            masks.append(mask_ext)
            nc.vector.tensor_scalar(mask_ext[:, 0:C + 1], iotatok[:, 0:C + 1],
                                    lensb[:, r:r + 1], None, Alu.is_lt)
            nc.gpsimd.memset(mask_ext[0:1, C:C + 1], 1.0)

        # ================= main MLP, two rows zippered =====================
        def main_pair(r0):
            rows = [r0, r0 + 1]
            hs = {}
            for i, r in enumerate(rows):
                hs[r] = (sp.tile([128, Cmax * CH], bf16, tag="h_a", bufs=2, name="h1"),
                         sp.tile([128, Cmax * CH], bf16, tag="h_b", bufs=2, name="h2"),
                         sp.tile([128, Cmax * CH], bf16, tag="h_c", bufs=2, name="p1"),
                         sp.tile([128, Cmax * CH], bf16, tag="h_d", bufs=2, name="p2"))
                Tp = Cs[r] * CH
                nc.gpsimd.memset(xTs[r][:, Tp:Tp + CH], 0.0)
                nc.scalar.copy(xTs[r][:, Tp:Tp + 1], demo_encT[:, r:r + 1])

            def layer_k(w, b, rhs_tile, rhs_rows, out_tile, k, Tp):
                N2 = min(1024, Tp - k * 1024)
                ps = pp.tile([128, 1024], f32, tag="mlp", bufs=2, name="ps_mlp")
                for ho in range(0, N2, 512):
                    N = min(512, N2 - ho)
                    nc.tensor.matmul(ps[:, ho:ho + N], w,
                                     rhs_tile[0:rhs_rows, k * 1024 + ho:k * 1024 + ho + N])
                evac_relu(out_tile[:, k * 1024:k * 1024 + N2], ps[:, 0:N2], b)

            NKs = {r: (Cs[r] * CH + 1023) // 1024 for r in rows}
            for k in range(max(NKs.values())):
                for r in rows:
                    if k < NKs[r]:
                        layer_k(w_phi1e, None, xTs[r], 33, hs[r][0], k, Cs[r] * CH)
                        layer_k(w_psi1e, None, xTs[r], 33, hs[r][2], k, Cs[r] * CH)
            for k in range(max(NKs.values())):
                for r in rows:
                    if k < NKs[r]:
                        layer_k(w_phi2, b_phi2, hs[r][0], 128, hs[r][1], k, Cs[r] * CH)
                        layer_k(w_psi2, b_psi2, hs[r][2], 128, hs[r][3], k, Cs[r] * CH)

            segs, ptoks, encd = {}, {}, {}
            for r in rows:
                enc_ext = sp.tile([128, Cs[r] + 1, 129], bf16, tag=f"enc{r}", name="enc_ext")
                encs.append(enc_ext)
                encd[r] = enc_ext
                segs[r] = mA[:, 264:265] if r % 2 == 0 else mS[:, 0:1]
                ptoks[r] = [None] * ((Cs[r] + 7) // 8)

            def seg_mms(r, g):
                c0 = g * 8
                nch = min(8, Cs[r] - c0)
                for j in range(nch):
                    c = c0 + j
                    nc.tensor.matmul(segs[r][:], ptoks[r][g][:, j, :],
                                     masks[r][:, c:c + 1],
                                     start=(c == 0), stop=(c == Cs[r] - 1))

            NG8s = {r: (Cs[r] + 7) // 8 for r in rows}
            for g in range(max(NG8s.values())):
                for r in rows:
                    if g >= NG8s[r]:
                        continue
                    c0 = g * 8
                    nch = min(8, Cs[r] - c0)
                    l3 = pp.tile([128, 8, 128], f32, tag="mlp", bufs=2, name="l3_phi")
                    for j in range(nch):
                        nc.tensor.matmul(l3[:, j, :],
                                         hs[r][1][:, (c0 + j) * CH:(c0 + j + 1) * CH], w_phi3)
                    evac_relu(encd[r][:, c0:c0 + nch, 0:128], l3[:, 0:nch, :], b_phi3)
                    l3p = pp.tile([128, 8, 128], f32, tag="mlp", bufs=2, name="l3_psi")
                    for j in range(nch):
                        nc.tensor.matmul(l3p[:, j, :],
                                         hs[r][3][:, (c0 + j) * CH:(c0 + j + 1) * CH], w_psi3)
                    ptok = sp.tile([128, 8, 128], bf16, tag="ptok", bufs=2, name="ptok")
                    ptoks[r][g] = ptok
                    evac_relu(ptok[:, 0:nch, :], l3p[:, 0:nch, :], b_psi3)
                    if g >= 1:
                        seg_mms(r, g - 1)
            for r in rows:
                seg_mms(r, NG8s[r] - 1)

            for r in rows:
                C = Cs[r]
                enc_ext = encd[r]
                nc.gpsimd.memset(enc_ext[:, 0:C + 1, 128:129], 1.0)
                nc.gpsimd.memset(enc_ext[:, C, 0:128], 0.0)
                nc.sync.dma_start(out=enc_ext[0:1, C, 0:128], in_=enc_demo_tok[r:r + 1, :])
                agg_in = sp.tile([128, 1], f32, tag="aggin", bufs=2, name="agg_in")
                nc.vector.tensor_scalar(agg_in[:], segs[r][:], psi_demo[:, r:r + 1],
                                        recipL1[:, r:r + 1], Alu.add, Alu.mult)
                ps_a = mC[:, 464 + (r % 2):465 + (r % 2)]
                nc.tensor.matmul(ps_a[:], w_rattn, agg_in[:])
                agg2 = sp.tile([128, 1], f32, tag="agg2", bufs=2, name="agg2")
                nc.scalar.activation(agg2[:], ps_a[:], Act.Relu, bias=b_rattn)
                ps_cr = mC[0:1, 466 + (r % 2) * 4:470 + (r % 2) * 4]
                nc.tensor.matmul(ps_cr[:], agg2[:], Vagg)
                vxe = sp.tile([PHI_IN + 1, H], bf16, tag="vxe", bufs=NSLOTS, name="vxe")
                vxes.append(vxe)
                nc.vector.tensor_copy(vxe[0:PHI_IN, :], Vx)
                nc.vector.tensor_copy(vxe[PHI_IN:PHI_IN + 1, :], ps_cr[:])

        # ================= attention tails (hh(r) zipped with pre(r+1)) ====
        def pre_mms(r):
            C = Cs[r]
            pb = (r % 2) * 132
            pre = mA[:, pb:pb + 132]
            for c in range(C + 1):
                nc.tensor.matmul(pre[:, c * 4:c * 4 + 4],
                                 xTs[r][:, c * CH:(c + 1) * CH], vxes[r][:])
            return pre

        def exp_mask(r, pre):
            C = Cs[r]
            e_tok = sp.tile([128, (Cmax + 1) * 4], bf16, tag="etok", bufs=2, name="e_tok")
            nc.scalar.activation(e_tok[:, 0:(C + 1) * 4], pre[:, 0:(C + 1) * 4], Act.Exp)
            e_m = sp.tile([128, Cmax + 1, 4], bf16, tag="em", bufs=2, name="e_m")
            nc.vector.tensor_tensor(
                out=e_m[:, 0:C + 1, :],
                in0=e_tok[:, 0:(C + 1) * 4].rearrange("p (c h) -> p c h", h=4),
                in1=masks[r][:, 0:C + 1].unsqueeze(2).to_broadcast([128, C + 1, 4]),
                op=Alu.mult)
            return e_m

        def attn_tails():
            pre = pre_mms(0)
            for r in range(NSLOTS):
                C = Cs[r]
                e_m = exp_mask(r, pre)
                hb = (r % 2) * 132
                hh = mC[0:4, hb:hb + 132]
                # zipper: hh(r) chunk streams hide pre(r+1) weight loads
                Cn = Cs[r + 1] if r + 1 < NSLOTS else -1
                if r + 1 < NSLOTS:
                    pn = ((r + 1) % 2) * 132
                    pre = mA[:, pn:pn + 132]
                for c in range(max(C, Cn) + 1):
                    if c <= C:
                        nc.tensor.matmul(hh[:, 0:129], e_m[:, c, :], encs[r][:, c, :],
                                         start=(c == 0), stop=(c == C))
                    if c <= Cn:
                        nc.tensor.matmul(pre[:, c * 4:c * 4 + 4],
                                         xTs[r + 1][:, c * CH:(c + 1) * CH], vxes[r + 1][:])
                rz = sp.tile([4, 1], f32, tag="rz", bufs=2, name="rz")
                nc.vector.reciprocal(rz[:], hh[:, 128:129])
                hh_sb = sp.tile([4, 128], bf16, tag="hhsb", bufs=2, name="hh_sb")
                nc.vector.tensor_scalar(hh_sb[:], hh[:, 0:128], rz[:], None, Alu.mult)
                ps_tr = xtb[:, 1016 + (r % 2) * 4:1020 + (r % 2) * 4]
                nc.tensor.transpose(ps_tr[:, :], hh_sb[:], ident[0:H, 0:H])
                nc.vector.tensor_copy(feat_all[:, r, :], ps_tr[:, :])

        # ================= emission schedule ===============================
        for r in range(4):
            prepass(r)
        main_pair(0)
        prepass(4)
        prepass(5)
        main_pair(2)
        prepass(6)
        prepass(7)
        main_pair(4)
        main_pair(6)
        attn_tails()

        # ================= rho MLP over all 8 rows =========================
        ps_r1 = pp.tile([128, 1024], f32, tag="mlp", bufs=2, name="ps_r1")
        for h in range(H):
            nc.tensor.matmul(ps_r1[:, 0:NSLOTS], w_rho1[:, h, :], feat_all[:, :, h],
                             start=(h == 0), stop=(h == H - 1))
        r1 = sp.tile([128, NSLOTS], f32, tag="r1", name="r1")
        nc.scalar.activation(r1[:], ps_r1[:, 0:NSLOTS], Act.Relu, bias=b_rho1)
        ps_r2 = pp.tile([128, 1024], f32, tag="mlp", bufs=2, name="ps_r2")
        nc.tensor.matmul(ps_r2[:, 0:NSLOTS], w_rho2, r1[:])
        r2 = sp.tile([128, NSLOTS], f32, tag="r2", name="r2")
        nc.scalar.activation(r2[:], ps_r2[:, 0:NSLOTS], Act.Relu, bias=b_rho2)
        ps_r3 = mC[0:1, 474:474 + NSLOTS]
        nc.tensor.matmul(ps_r3[:], w_rho3, r2[:])
        # sigmoid(z) = 1/(1+exp(-z-b3)) stays within the exp table set
        en = sp.tile([1, NSLOTS], f32, tag="en", name="en")
        nc.scalar.activation(en[:], ps_r3[:], Act.Exp, bias=nb_rho3, scale=-1.0)
        ep1 = sp.tile([1, NSLOTS], f32, tag="ep1", name="ep1")
        nc.vector.tensor_scalar(ep1[:], en[:], 1.0, None, Alu.add)
        res = sp.tile([1, NSLOTS], f32, tag="res", name="res")
        nc.vector.reciprocal(res[:], ep1[:])
        nc.sync.dma_start(out=d_out[:].rearrange("r one -> one r"), in_=res[:])
    return nc


def _patch_tile_drain(tile_mod, mybir):
    """Walrus in this env rejects >1 sync wait per instruction."""
    from concourse.vector_clock import ScopedClock
    if getattr(tile_mod.TileContext, "_drain_patched", False):
        return

    def _drain_and_barrier(self, tick_clock, wait_clock):
        nc = self.nc
        drain_inst = nc.sync.drain()
        wait_clock.add_sem_waits(drain_inst.ins, ScopedClock({None: tick_clock.global_clock}))
        si = drain_inst.ins.sync_info
        waits = list(si.on_wait or [])
        if len(waits) > 1:
            si.on_wait = waits[:1]
            for i in range(1, len(waits)):
                extra = nc.sync.drain()
                esi = extra.ins.sync_info
                if esi is None:
                    extra.ins.sync_info = mybir.SyncInfo(on_wait=waits[i:i + 1], on_update=[])
                else:
                    esi.on_wait = waits[i:i + 1]
        nc.all_engine_barrier()
        popped = nc._tile_sem_poison_stack.pop()
        assert popped is self._sem_poison
        nc.clear_and_free_semaphores(list(self.sems.allocated().values()))
        nc.all_engine_barrier()

    tile_mod.TileContext._drain_and_barrier = _drain_and_barrier

    _orig_exit = tile_mod.TileContext.__exit__

    def _exit(self, exc_type, exc_val, exc_tb):
        r = _orig_exit(self, exc_type, exc_val, exc_tb)
        if exc_type is None and getattr(tile_mod.TileContext, "_split_waits", True):
            _split_multi_waits(self.nc, mybir)
        return r

    def _split_multi_waits(nc, mybir):
        n = [0]
        for f in nc.m.functions:
            for bb in f.blocks:
                insts = bb.instructions
                out = []
                for inst in insts:
                    si = inst.sync_info
                    waits = list(si.on_wait) if (si and si.on_wait) else []
                    if len(waits) > 1:
                        for w in waits[:-1]:
                            n[0] += 1
                            nop = mybir.InstNoOp(name=f"I-ws-{n[0]}", ins=[], outs=[])
                            nop.engine = inst.engine
                            nop.sync_info = mybir.SyncInfo(on_wait=[w], on_update=[])
                            out.append(nop)
                        si.on_wait = waits[-1:]
                    out.append(inst)
                if len(out) != len(insts):
                    bb.instructions = out

    tile_mod.TileContext.__exit__ = _exit
    tile_mod.TileContext._drain_patched = True


_CACHE = {}
last_results = None


def _maybe_install_ntff_shim():
    import sys, types
    if "antenv.axon_hooks" in sys.modules:
        return
    try:
        from trn_agent_boot.trn_boot import _ntff_profile_via_ctypes
        hook = _ntff_profile_via_ctypes("/opt/axon/libaxon_pjrt.so")
    except Exception:
        hook = None
    mod = types.ModuleType("antenv.axon_hooks")
    mod.get_axon_ntff_profile_hook = lambda: hook
    sys.modules["antenv.axon_hooks"] = mod


def _to_bf16(a):
    import ml_dtypes
    return np.asarray(a, np.float32).astype(ml_dtypes.bfloat16)


def _build_packs(inp):
    """Host-side packed constant tensors (one f32, one bf16).
    W_k @ W_q is folded on host into Vx [32,4] / Vagg [128,4]."""
    wq = inp["W_q"].astype(np.float32) * (1.0 / np.sqrt(DP))
    Wk = inp["W_k"].astype(np.float32).reshape(PHI_IN + 128, H, DP)
    V = np.einsum("fhd,hd->fh", Wk, wq)          # [160, 4]
    f = np.zeros((128, NF), np.float32)
    f[:, F_ID32:F_ID32 + 32] = np.tile(np.eye(32, dtype=np.float32), (4, 1))
    f[:, F_ITP8:F_ITP8 + 8] = (1.0 / (MAXTS ** np.linspace(0.0, 1.0, NPOS // 2)
                                      .astype(np.float32) * TWOPI))[None, :]
    f[:, F_IOTA15:F_IOTA15 + NMOD] = np.arange(NMOD, dtype=np.float32)[None, :]
    f[:, F_IOTATOK:F_IOTATOK + 33] = (np.arange(128, dtype=np.float32)[:, None]
                                      + 128.0 * np.arange(33, dtype=np.float32)[None, :])
    f[:, F_BD1] = inp["demo_b1"]
    f[:, F_BP2] = inp["phi_b2"]
    f[:, F_BP3] = inp["phi_b3"]
    f[:, F_BS2] = inp["psi_b2"]
    f[:, F_BS3] = inp["psi_b3"]
    f[:, F_BRA] = inp["rho_attn_b"]
    f[:, F_BR1] = inp["rho_b1"]
    f[:, F_BR2] = inp["rho_b2"]
    f[0:32, F_BD2] = inp["demo_b2"]
    f[0, F_BNR3] = -inp["rho_b3"][0]
    f[:, F_VAGG:F_VAGG + 4] = V[PHI_IN:]
    f[:, F_RATTW:F_RATTW + 128] = inp["rho_attn_w"]
    f[:, F_RW2:F_RW2 + 128] = inp["rho_w2"]
    f[:, F_RW3:F_RW3 + 1] = inp["rho_w3"]

    g = np.zeros((128, NB), np.float32)
    g[:, G_IDENT:G_IDENT + 128] = np.eye(128, dtype=np.float32)
    g[0:33, G_W1A:G_W1A + 128] = np.concatenate(
        [inp["phi_w1"], inp["phi_b1"][None, :]], axis=0)
    g[64:72, G_W1A:G_W1A + 128] = inp["demo_w1"]
    g[0:33, G_W1B:G_W1B + 128] = np.concatenate(
        [inp["psi_w1"], inp["psi_b1"][None, :]], axis=0)
    g[:, G_DW2:G_DW2 + 32] = inp["demo_w2"]
    g[0:PHI_IN, G_VX:G_VX + 4] = V[0:PHI_IN]
    g[:, G_PW2:G_PW2 + 128] = inp["phi_w2"]
    g[:, G_SW2:G_SW2 + 128] = inp["psi_w2"]
    g[:, G_PW3:G_PW3 + 128] = inp["phi_w3"]
    g[:, G_SW3:G_SW3 + 128] = inp["psi_w3"]
    g[:, G_RW1:G_RW1 + 512] = inp["rho_w1"].reshape(4, 128, 128).transpose(1, 0, 2).reshape(128, 512)
    return f, _to_bf16(g)


def _prepare(inputs):
    import concourse.bass as bass
    import concourse.mybir as mybir
    import concourse.tile as tile_mod

    _patch_tile_drain(tile_mod, mybir)

    inp = {k: np.asarray(v, np.float32) if np.asarray(v).dtype != np.float32 else np.asarray(v)
           for k, v in inputs.items()}
    times = np.ascontiguousarray(inp["times"][..., 0])    # [B,T]
    values = np.ascontiguousarray(inp["values"][..., 0])  # [B,T]
    meas = np.ascontiguousarray(inputs["measurements"]).astype(np.float32)
    lengths = np.asarray(inputs["lengths"]).astype(np.int64)
    demo = inp["demo"]

    order = np.argsort(-lengths, kind="stable")
    Cs = []
    for s in range(NSLOTS):
        ranks = order[s * NCORES:(s + 1) * NCORES]
        Cs.append(int(np.ceil(lengths[ranks].max() / CH)))

    key = tuple(Cs)
    if key not in _CACHE:
        _CACHE[key] = _build_nc(Cs, tile_mod, bass, mybir)
    nc = _CACHE[key]

    cpf, cpb = _build_packs(inp)

    in_maps = []
    for core in range(NCORES):
        rows = [order[s * NCORES + core] for s in range(NSLOTS)]
        m = {
            "times_r": times[rows].reshape(NSLOTS * CMAXG, CH),
            "values_r": values[rows].reshape(NSLOTS * CMAXG, CH),
            "meas_r": meas[rows].reshape(NSLOTS * CMAXG, CH),
            "demo_r": _to_bf16(demo[rows]),
            "lens_bcast": np.ascontiguousarray(
                np.broadcast_to(lengths[rows].astype(np.float32)[None, :], (128, NSLOTS))),
            "cpf": cpf, "cpb": cpb,
        }
        in_maps.append(m)
    return nc, in_maps, order


def kernel(**inputs):
    import os
    from concourse import bass_utils

    nc, in_maps, order = _prepare(inputs)

    trace = os.environ.get("KERNEL_TRACE", "0") == "1"
    kw = {}
    if trace:
        _maybe_install_ntff_shim()
        kw = dict(trace=True, tmpdir=os.environ.get("KERNEL_TRACE_DIR") or None)
    res = bass_utils.run_bass_kernel_spmd(nc, in_maps, core_ids=list(range(NCORES)), **kw)
    global last_results
    last_results = res
    out = np.zeros((B, 1), np.float32)
    for core in range(NCORES):
        for s in range(NSLOTS):
            out[order[s * NCORES + core], 0] = res.results[core]["out"][s, 0]
    return out
